# revision 52
# baseline (speedup 1.0000x reference)
import sys

sys.path.insert(0, "/opt/trn_rl_repo")

import numpy as np
import ml_dtypes

import concourse.bacc as bacc
import concourse.bass as bass
import concourse.mybir as mybir
import concourse.tile as tile
from concourse.bass_utils import run_bass_kernel_spmd

F32 = mybir.dt.float32
BF16 = mybir.dt.bfloat16
AF = mybir.ActivationFunctionType
ALU = mybir.AluOpType
AX = mybir.AxisListType

# Problem constants (hardcoded per harness contract).
B, C, H, W = 4, 64, 128, 128
NT = 9          # 3x3 taps
NFF = 4         # factor*factor subpixels
NCORES = 8
HL = H // 2     # 64 coarse rows per core
NYB = 4         # y-blocks
YB = HL // NYB  # 16 rows per block
N = YB * C      # per-tap product free elems per partition: (yl, c)
POOL_TAPS = (1, 4)   # taps whose products run on the Pool (gpsimd) engine

_cached = {}


def ap_of(t, off, dims):
    base = t[:]
    return bass.AP(base.tensor, base.offset + off, dims)


def build_nc():
    nc = bacc.Bacc("TRN2", target_bir_lowering=False, debug=False, num_devices=NCORES)

    hp2_d = nc.dram_tensor("hp2", [128, NYB * 18 * 130], BF16, kind="ExternalInput")
    hp4_d = nc.dram_tensor("hp4", [128, NYB * 18 * 130], BF16, kind="ExternalInput")
    hT_d = nc.dram_tensor("hT", [128, NYB * 3 * 18 * C], BF16, kind="ExternalInput")
    w1a_d = nc.dram_tensor("w1a", [128, 3 * 128], BF16, kind="ExternalInput")
    w4a_d = nc.dram_tensor("w4a", [128, 128], BF16, kind="ExternalInput")
    w4b_d = nc.dram_tensor("w4b", [64, 128], BF16, kind="ExternalInput")
    b1_d = nc.dram_tensor("b1c", [128, 1], F32, kind="ExternalInput")
    w2t_d = nc.dram_tensor("w2t", [128, 36], BF16, kind="ExternalInput")
    b2_d = nc.dram_tensor("b2c", [36, 1], F32, kind="ExternalInput")
    idb_d = nc.dram_tensor("idb", [128, 128], BF16, kind="ExternalInput")
    # out layout: [x, (ff, yb, k, yl8, c64)] bf16
    out_d = nc.dram_tensor("out", [128, NFF * HL * C], BF16, kind="ExternalOutput")

    with tile.TileContext(nc) as tc:
        with (
            tc.tile_pool(name="const", bufs=1) as cpool,
            tc.tile_pool(name="ring", bufs=2) as ring,
            tc.tile_pool(name="mchunk", bufs=3) as mpool,
            tc.tile_pool(name="prodp", bufs=3) as prodp,
            tc.tile_pool(name="obuf", bufs=3) as opool,
            tc.tile_pool(name="ps1", bufs=2, space=bass.MemorySpace.PSUM) as pp1,
            tc.tile_pool(name="ps2", bufs=1, space=bass.MemorySpace.PSUM) as pp2,
            tc.tile_pool(name="pst", bufs=1, space=bass.MemorySpace.PSUM) as ppt,
            tc.tile_pool(name="psw", bufs=3, space=bass.MemorySpace.PSUM) as ppw,
        ):
            # ---- constants; conv weights first so conv can start asap ----
            w1a = cpool.tile([128, 3 * 128], BF16)
            w4a = cpool.tile([128, 128], BF16)
            w4b = cpool.tile([64, 128], BF16)
            b1 = cpool.tile([128, 1], F32)
            w2t = cpool.tile([128, 36], BF16)
            b2 = cpool.tile([36, 1], F32)
            idb = cpool.tile([128, 128], BF16)
            # per-yb input windows (18 rows each, host-duplicated halo rows).
            # yb0's are split in half-windows so conv starts after ~0.4MB of DMA.
            hp2y = [None] + [cpool.tile([128, 18 * 130], BF16, name=f"hp2y{i}")
                             for i in range(1, NYB)]
            hp4y = [None] + [cpool.tile([128, 18 * 130], BF16, name=f"hp4y{i}")
                             for i in range(1, NYB)]
            hp2y0 = [cpool.tile([128, 11 * 130], BF16, name="hp2y0a"),
                     cpool.tile([128, 10 * 130], BF16, name="hp2y0b")]
            hp4y0 = [cpool.tile([128, 10 * 130], BF16, name="hp4y0a"),
                     cpool.tile([128, 10 * 130], BF16, name="hp4y0b")]
            hTy = [cpool.tile([128, 3 * 18 * C], BF16, name=f"hTy{i}")
                   for i in range(NYB)]
            nc.sync.dma_start(w1a[:], w1a_d[:])
            nc.sync.dma_start(w4a[:], w4a_d[:])
            nc.sync.dma_start(w4b[:], w4b_d[:])
            nc.sync.dma_start(b1[:], b1_d[:])
            nc.sync.dma_start(hp2y0[0][:], hp2_d[:, 0 : 11 * 130])
            nc.sync.dma_start(hp4y0[0][:], hp4_d[:, 0 : 10 * 130])
            nc.sync.dma_start(hp2y0[1][:], hp2_d[:, 8 * 130 : 18 * 130])
            nc.sync.dma_start(hp4y0[1][:], hp4_d[:, 8 * 130 : 18 * 130])
            nc.sync.dma_start(w2t[:], w2t_d[:])
            nc.sync.dma_start(b2[:], b2_d[:])
            nc.sync.dma_start(idb[:], idb_d[:])
            nc.sync.dma_start(hTy[0][:], hT_d[:, 0 : 3 * 18 * C])
            for i in range(1, NYB):
                nc.sync.dma_start(hp2y[i][:],
                                  hp2_d[:, i * 18 * 130 : (i + 1) * 18 * 130])
                nc.sync.dma_start(hp4y[i][:],
                                  hp4_d[:, i * 18 * 130 : (i + 1) * 18 * 130])
                nc.sync.dma_start(hTy[i][:],
                                  hT_d[:, i * 3 * 18 * C : (i + 1) * 3 * 18 * C])

            def mask_phase(yb, halves=False):
                """Returns (state, pieces): closures emitting the mask branch
                for block yb, software-pipelined so PE never waits on Act."""
                st = {"ms": []}

                def start():
                    st["eb"] = ring.tile([36, 4 * 512], BF16, name="eb", tag="eb")
                    st["eTb"] = ring.tile([128, YB * 40], BF16, name="eTb", tag="eTb")
                    st["rzT"] = ring.tile([128, YB * 4], BF16, name="rzT", tag="rzT")

                def conv1(ic):
                    if yb == 0:
                        t2, t4 = hp2y0[ic // 2], hp4y0[ic // 2]
                        t2n, t4n = (11, 10) if ic < 2 else (10, 10)
                        rb = 0 if ic < 2 else 8
                    else:
                        t2, t4, t2n, t4n, rb = hp2y[yb], hp4y[yb], 18, 18, 0
                    ps1 = pp1.tile([128, 512], F32)
                    for dy in range(3):
                        rhs = ap_of(t2, (4 * ic + dy - rb) * 130,
                                    [[t2n * 130, 128], [130, 4], [1, 128]])
                        nc.tensor.matmul(ps1[:], w1a[:, dy * 128:(dy + 1) * 128], rhs,
                                         start=(dy == 0), stop=False)
                    # taps (0,2)+(1,2) in one matmul via dual-row packing, then (2,2)
                    rhs = ap_of(t4, (4 * ic - rb) * 130,
                                [[t4n * 130, 128], [130, 4], [1, 128]])
                    nc.tensor.matmul(ps1[:], w4a[:], rhs, start=False, stop=False)
                    rhs = ap_of(t4, (4 * ic + 2 - rb) * 130,
                                [[t4n * 130, 64], [130, 4], [1, 128]])
                    nc.tensor.matmul(ps1[:], w4b[:], rhs, start=False, stop=True)
                    m = mpool.tile([128, 512], BF16, tag="m")
                    nc.scalar.activation(m[:], ps1[:], AF.Relu, bias=b1[:], scale=1.0)
                    st["ms"].append(m)

                def conv2(ic):
                    eb = st["eb"]
                    ps2 = pp2.tile([36, 512], F32)
                    nc.tensor.matmul(ps2[:], w2t[:], st["ms"][ic][:])
                    nc.scalar.activation(eb[:, ic * 512:(ic + 1) * 512],
                                         ps2[:], AF.Exp, bias=b2[:], scale=1.0)

                def etrans_e(j):
                    # e transposes for rows of chunk j -> eTb [x, (yl, 0:36)]
                    eb, eTb = st["eb"], st["eTb"]
                    pst = ppt.tile([128, 4 * 36], BF16, name="pst")
                    for r in range(4):
                        yl = j * 4 + r
                        nc.tensor.transpose(pst[:, r * 36:(r + 1) * 36],
                                            eb[:, yl * 128:(yl + 1) * 128],
                                            idb[0:36, 0:36])
                    o_ap = ap_of(eTb, j * 160,
                                 [[YB * 40, 128], [40, 4], [1, 36]])
                    nc.scalar.copy(o_ap, pst[:])

                def zn(r0, nr):
                    # Z = per-ff sum of 9 e's for rows [r0, r0+nr), 1/Z, then
                    # normalized mask rows into nmb (transposed+duplicated bf16)
                    eTb, rzT, zT, nmb = st["eTb"], st["rzT"], st["zT"], st["nmb"]
                    in_ap = ap_of(eTb, r0 * 40,
                                  [[YB * 40, 128], [40, nr], [9, 4], [1, 9]])
                    z_ap = ap_of(zT, r0 * 4, [[YB * 4, 128], [4, nr], [1, 4]])
                    nc.vector.tensor_reduce(z_ap, in_ap, AX.X, ALU.add)
                    with nc.allow_low_precision(reason="1/Z bf16 @2e-2 tol"):
                        nc.vector.reciprocal(rzT[:, r0 * 4:(r0 + nr) * 4],
                                             zT[:, r0 * 4:(r0 + nr) * 4])
                    for ff in range(NFF):
                        out_ap = ap_of(nmb, ff * 18 + r0 * 72,
                                       [[YB * 72, 128], [72, nr], [2, 9], [1, 2]])
                        in0 = ap_of(eTb, ff * 9 + r0 * 40,
                                    [[YB * 40, 128], [40, nr], [1, 9], [0, 2]])
                        in1 = ap_of(rzT, ff + r0 * 4,
                                    [[YB * 4, 128], [4, nr], [0, 9], [0, 2]])
                        nc.vector.tensor_tensor(out_ap, in0, in1, ALU.mult)

                def start():
                    st["eb"] = ring.tile([36, 4 * 512], BF16, name="eb", tag="eb")
                    st["eTb"] = ring.tile([128, YB * 40], BF16, name="eTb", tag="eTb")
                    st["rzT"] = ring.tile([128, YB * 4], BF16, name="rzT", tag="rzT")
                    st["zT"] = ring.tile([128, YB * 4], F32, name="zT", tag="zT")
                    st["nmb"] = ring.tile([128, YB * 72], BF16, name="nmb", tag="nmb")
                st["zn"] = zn
                st["start"] = start

                if halves:
                    pieces = [
                        lambda: (start(), conv1(0)),
                        lambda: conv1(1),
                        lambda: conv2(0),
                        lambda: (conv2(1), etrans_e(0)),
                        lambda: etrans_e(1),
                        lambda: zn(0, 8),
                        lambda: conv1(2),
                        lambda: conv1(3),
                        lambda: conv2(2),
                        lambda: (conv2(3), etrans_e(2)),
                        lambda: etrans_e(3),
                        lambda: zn(8, 8),
                    ]
                else:
                    pieces = [
                        lambda: (start(), conv1(0)),
                        lambda: conv1(1),
                        lambda: conv2(0),
                        lambda: conv1(2),
                        lambda: (conv2(1), etrans_e(0)),
                        lambda: conv1(3),
                        lambda: (conv2(2), etrans_e(1)),
                        lambda: (conv2(3), etrans_e(2)),
                        lambda: etrans_e(3),
                        lambda: zn(0, YB),
                    ]
                return st, pieces

            def products(yb, st, ff, r0=0, nr=YB):
                pn = nr * C
                prod = prodp.tile([128, NT * pn], BF16, name="prod",
                                  tag="prod" if nr == YB else "prodh")
                for dy in range(3):
                    for dx in range(3):
                        t = dy * 3 + dx
                        in0 = ap_of(hTy[yb], (dx * 18 + r0 + dy) * C,
                                    [[3 * 18 * C, 128], [C, nr], [2, 32], [1, 2]])
                        in1 = ap_of(st["nmb"], ff * 18 + t * 2 + r0 * 72,
                                    [[YB * 72, 128], [72, nr], [0, 32], [1, 2]])
                        po = ap_of(prod, t * pn,
                                   [[NT * pn, 128], [C, nr], [2, 32], [1, 2]])
                        eng = nc.gpsimd if t in POOL_TAPS else nc.vector
                        eng.tensor_tensor(po, in0, in1, ALU.mult)
                return prod, pn

            def accum_one(yb, ff, prod, pn, k, koff):
                psw = ppw.tile([128, 512], F32)
                for t in range(NT):
                    nc.tensor.matmul(
                        psw[:], idb[:],
                        prod[:, t * pn + k * 512 : t * pn + (k + 1) * 512],
                        start=(t == 0), stop=(t == NT - 1))
                ob = opool.tile([128, 512], BF16)
                nc.scalar.copy(ob[:], psw[:])
                nc.sync.dma_start(
                    out_d[:, ((ff * NYB + yb) * 2 + koff) * 512 :
                             ((ff * NYB + yb) * 2 + koff + 1) * 512],
                    ob[:])

            # interleaved emission: while DVE computes products(yb, ff), PE
            # runs mask pieces of yb+1; then PE accumulates taps of (yb, ff).
            # yb0's mask runs in half-blocks so DVE starts as early as possible.
            sts = [None] * NYB
            sts[0], p0 = mask_phase(0, halves=True)
            for p in p0[0:6]:
                p()
            sts[1], p1 = mask_phase(1)
            g0 = [p0[6:8], p0[8:10], p0[10:11], p0[11:12]]
            for ff in range(NFF):
                prod, pn = products(0, sts[0], ff, r0=0, nr=8)
                for p in g0[ff]:
                    p()
                accum_one(0, ff, prod, pn, 0, 0)
            g1 = [p1[0:2], p1[2:4], p1[4:6], p1[6:10]]
            for ff in range(NFF):
                prod, pn = products(0, sts[0], ff, r0=8, nr=8)
                for p in g1[ff]:
                    p()
                accum_one(0, ff, prod, pn, 0, 1)
            for yb in range(1, NYB):
                nxt = []
                if yb + 1 < NYB:
                    sts[yb + 1], nxt = mask_phase(yb + 1)
                groups = [nxt[0:3], nxt[3:6], nxt[6:9], nxt[9:10]]
                for ff in range(NFF):
                    prod, pn = products(yb, sts[yb], ff)
                    for p in groups[ff]:
                        p()
                    accum_one(yb, ff, prod, pn, 0, 0)
                    accum_one(yb, ff, prod, pn, 1, 1)

    nc.compile()
    return nc


def prep_shared(W1, b1, W2, b2):
    W1 = np.asarray(W1, np.float32)
    b1 = np.asarray(b1, np.float32)
    W2 = np.asarray(W2, np.float32).reshape(36, 128)
    b2 = np.asarray(b2, np.float32)

    w1a = np.zeros((128, 3 * 128), np.float32)
    for dy in range(3):
        w1a[0:64, dy * 128:(dy + 1) * 128] = W1[:, :, dy, 0].T
        w1a[64:128, dy * 128:(dy + 1) * 128] = W1[:, :, dy, 1].T
    w4a = np.concatenate([W1[:, :, 0, 2].T, W1[:, :, 1, 2].T], axis=0)  # [128,128]
    w4b = W1[:, :, 2, 2].T  # [64, 128]

    o_of_mp = np.array([t * 4 + ff for ff in range(4) for t in range(9)])
    w2t = np.ascontiguousarray((0.25 * W2[o_of_mp, :]).T)
    b2c = np.ascontiguousarray((0.25 * b2[o_of_mp]).reshape(36, 1))

    return {
        "w1a": w1a.astype(ml_dtypes.bfloat16),
        "w4a": w4a.astype(ml_dtypes.bfloat16),
        "w4b": w4b.astype(ml_dtypes.bfloat16),
        "b1c": b1.reshape(128, 1).astype(np.float32),
        "w2t": w2t.astype(ml_dtypes.bfloat16),
        "b2c": b2c.astype(np.float32),
        "idb": np.eye(128, dtype=ml_dtypes.bfloat16),
    }


def kernel(h, W1, b1, W2, b2, _trace=False):
    h = np.asarray(h, np.float32)
    shared = prep_shared(W1, b1, W2, b2)

    hp = np.pad(h, ((0, 0), (0, 0), (1, 1), (1, 1)))  # [B, C, 130, 130]
    in_maps = []
    for core in range(NCORES):
        b, half = core // 2, core % 2
        y0 = half * HL
        win = hp[b, :, y0:y0 + 66, :]  # [64c, 66y, 130x] rows y0-1..y0+64 padded
        hp2 = np.zeros((128, 66, 130), np.float32)
        hp2[0:64] = win
        hp2[64:128, :, 0:129] = win[:, :, 1:130]
        # per-yb overlapping 18-row windows: rows 16*yb .. 16*yb+17
        hp2s = np.stack([hp2[:, 16 * i:16 * i + 18, :] for i in range(NYB)], axis=1)
        # hp4: [c, dx2@y | c, dx2@y+1] packing (row 17 lower half unused)
        win67 = np.concatenate([win, np.zeros((64, 1, 130), np.float32)], axis=1)
        hp4 = np.zeros((128, 66, 130), np.float32)
        hp4[0:64, :, 0:128] = win67[:, 0:66, 2:130]
        hp4[64:128, :, 0:128] = win67[:, 1:67, 2:130]
        hp4s = np.stack([hp4[:, 16 * i:16 * i + 18, :] for i in range(NYB)], axis=1)
        # hT[x][(yb, dx, yrows18, c)] = 8*win[c, y, x+dx]
        h8 = 8.0 * win
        sh = np.empty((128, 3, 66, C), np.float32)
        for dx in range(3):
            sh[:, dx] = h8[:, :, dx:dx + 128].transpose(2, 1, 0)  # [x, y66, c]
        hTf = np.stack([sh[:, :, 16 * i:16 * i + 18, :] for i in range(NYB)], axis=1)
        m = dict(shared)
        m["hp2"] = hp2s.reshape(128, -1).astype(ml_dtypes.bfloat16)
        m["hp4"] = hp4s.reshape(128, -1).astype(ml_dtypes.bfloat16)
        m["hT"] = hTf.reshape(128, -1).astype(ml_dtypes.bfloat16)
        in_maps.append(m)

    if "nc" not in _cached:
        _cached["nc"] = build_nc()
    res = run_bass_kernel_spmd(_cached["nc"], in_maps, core_ids=list(range(NCORES)),
                               trace=_trace)

    out = np.zeros((B, C, 2 * H, 2 * W), np.float32)
    for core in range(NCORES):
        b, half = core // 2, core % 2
        # res: [x, (ff, yb, k, yl8, c)] -> out[b, c, 2y+fy, 2x+fx]
        r = np.asarray(res.results[core]["out"], np.float32)
        r = r.reshape(128, NFF, HL, C)          # [x, ff, y, c]
        r = r.transpose(3, 1, 2, 0)             # [c, ff, y, x]
        r = r.reshape(C, 2, 2, HL, W)           # [c, fy, fx, y, x]
        r = r.transpose(0, 3, 1, 4, 2)          # [c, y, fy, x, fx]
        out[b, :, half * 128:(half + 1) * 128, :] = r.reshape(C, 128, 256)
    if _trace:
        return out, res
    return out
